# revision 48
# baseline (speedup 1.0000x reference)
"""Linformer-style multihead attention on 8 Trainium2 NeuronCores.

Shapes (hardcoded): B=4, S=8192, D=512, H=8, DK=DV=64, PK=256.

Sharding: core c handles batch b=c//2, sequence half h=c%2 (4096 query rows).
The Linformer K/V projections contract over the FULL sequence; each core
contracts its own half and the batch-pair AllReduces the tiny khT/vh partials.

Key algebra (reassociation): reference computes k = value@Wk then We^T@k.
We instead compute VP^T = value^T@We (8192-contraction, emitted feature-major
directly so no PE transposes are needed) then khT = Wk^T@VP^T (512-
contraction). Biases fold in as rank-1 augmentation rows of the small
matmuls; bq rides the q-projection drain; bo is applied on the host.

Schedule: ~10 warmup matmuls on zeros keep the PE HAM clock-gate warm from
t~1us (cold PE runs at 1.2GHz vs 2.4 warm). Phase B streams one host-packed
[v|we|wf] tensor in 4x2MB blocks (16KB partition lines) alone on the sync
HWDGE queue; weights ride one packed blob + host-pretransposed q on scalar
(few, big transfers: the ~8 recycled DMA semaphores are shared across
queues, so many small triggers false-serialize cross-queue; the on-chip
x-bar DMA-transpose fragments to ~330B packets and starves everything).
kh/vh partials feed TWO pair-AllReduces (khT first, then vh; each ~10-12us
data vs ~32us combined); the q projection (two chains pulled early to fill
the vpf-drain stall) hides the first, and the attention scores+exp
pipeline - which needs only khT - runs LAG=8 groups ahead of AV/Z so the
PE works through the second. Attention runs per
(s-tile, head-pair, pk-chunk) groups: scores row-paired on the PE, exp
batched as one N=1024 scalar activation spanning two PSUM banks, AV +
denominator col-paired, softmax normalize and PSUM drains on the vector
engine, output stores in bf16. Everything PE-side is bf16.
"""

import numpy as np
import ml_dtypes
from contextlib import ExitStack

import concourse.bass as bass
import concourse.bacc as bacc
import concourse.mybir as mybir
import concourse.tile as tile
from concourse import bass_utils

B, S, D = 4, 8192, 512
H, DK, DV, PK = 8, 64, 64, 256
SH = S // 2  # per-core query rows
NCORES = 8
P = 128

F32 = mybir.dt.float32
BF16 = mybir.dt.bfloat16
AF = mybir.ActivationFunctionType
OP = mybir.AluOpType

_CACHE = {}

NB = 4          # phase-B n-blocks of 1024 rows
RB = 8          # contiguous rows per partition line
NT = SH // 512  # 8 s-tiles of 512 query rows


def _build_kernel():
    nc = bacc.Bacc(
        trn_type="TRN2",
        target_bir_lowering=False,
        debug=False,
        num_devices=NCORES,
    )

    # vwef: host-packed [v | we | wf] rows (1024 cols) so each phase-B block
    # is ONE 2MB DMA; wall: all [P,4,D]-layout weights (wq,wk,wv,wo) packed;
    # aug: the 4 rank-augmentation row-pairs packed.
    q_t = nc.dram_tensor("q", [D, SH], BF16, kind="ExternalInput").ap()
    vwef_t = nc.dram_tensor("vwef", [SH, D + 2 * PK], BF16,
                            kind="ExternalInput").ap()
    wall_t = nc.dram_tensor("wall", [P, 16, D], BF16, kind="ExternalInput").ap()
    aug_t = nc.dram_tensor("aug", [2, 2 * D + 2 * PK], BF16,
                           kind="ExternalInput").ap()
    bq_t = nc.dram_tensor("bq", [D], F32, kind="ExternalInput").ap()
    out_t = nc.dram_tensor("out", [SH, D], BF16, kind="ExternalOutput").ap()

    with ExitStack() as ctx:
        tc = ctx.enter_context(tile.TileContext(nc))
        consts = ctx.enter_context(tc.tile_pool(name="consts", bufs=1))
        big = ctx.enter_context(tc.tile_pool(name="big", bufs=1))

        # ---- persistent activations ----
        qTraw = big.tile([P, 4, SH], BF16)     # query, feature-major
        khT = big.tile([P, 4, PK], BF16)       # [dk(2 heads/row-block), pair, pk]
        vh_sb = big.tile([P, 2, H, DV], BF16)  # [pk rows, chunk, head, dv]
        vpf_sb = big.tile([P, 4, 2, PK], BF16)  # [e-in-slice, e-slice, vp|vf, pk]
        qsts = [big.tile([P, 4, 512], BF16, name=f"qst{st}") for st in range(NT)]

        # ---- constants / weights ----
        wall_sb = consts.tile([P, 16, D], BF16)
        wq_sb = wall_sb[:, 0:4, :]
        wk_sb = wall_sb[:, 4:8, :]
        wv_sb = wall_sb[:, 8:12, :]
        wo_sb = wall_sb[:, 12:16, :]
        aug_sb = consts.tile([2, 2 * D + 2 * PK], BF16)
        wkaug_sb = aug_sb[:, 0:D]
        auge_sb = aug_sb[:, D:D + PK]
        wvaug_sb = aug_sb[:, D + PK:2 * D + PK]
        augf_sb = aug_sb[:, 2 * D + PK:2 * D + 2 * PK]
        bq_sb = consts.tile([P, 4], F32)
        ones64 = consts.tile([P, 64], BF16)
        zeros_sb = consts.tile([P, 512], BF16)
        nc.gpsimd.memset(ones64, 1.0)
        nc.gpsimd.memset(zeros_sb, 0.0)

        # ---- DMA kickoff (all streams fire immediately) ----
        # Packed phase-B stream alone on sync (4 x 2MB, 16KB partition
        # lines); weights + host-pretransposed q halves on scalar. Few, big
        # transfers: the ~8 recycled DMA semaphore slots are shared across
        # queues, so many small triggers false-serialize cross-queue.
        vwef_r = vwef_t.rearrange("(n p r) d -> p n (r d)", p=P, r=RB)
        W = D + 2 * PK
        vwp_cm = tc.tile_pool(name="vwp", bufs=1)
        vwp = vwp_cm.__enter__()
        vwefs = []
        # blocks 0-2 on sync; block 3 on scalar (after the weights, before
        # q): the rings drain in parallel so the LAST-consumed block is
        # already resident when the sync stream finishes - phase-B's matmul
        # end moves ~8us earlier. Both rings stay continuously busy (an idle
        # ring with deferred q traffic is the known-bad collective pattern).
        for n in range(NB):
            t = vwp.tile([P, RB, W], BF16, name=f"vwef{n}")
            vwefs.append(t)
        for n in range(NB - 1):
            f = vwefs[n].rearrange("p r d -> p (r d)")
            step = RB * W // 4
            for k in range(4):
                nc.sync.dma_start(out=f[:, k * step:(k + 1) * step],
                                  in_=vwef_r[:, n, k * step:(k + 1) * step])
        nc.scalar.dma_start(out=wall_sb, in_=wall_t)
        nc.scalar.dma_start(out=aug_sb, in_=aug_t)
        nc.scalar.dma_start(out=bq_sb, in_=bq_t.rearrange("(c p) -> p c", p=P))
        f3 = vwefs[NB - 1].rearrange("p r d -> p (r d)")
        step = RB * W // 2
        for k in range(2):
            nc.scalar.dma_start(out=f3[:, k * step:(k + 1) * step],
                                in_=vwef_r[:, NB - 1, k * step:(k + 1) * step])
        # q is pre-transposed on the host (feature-major); the on-chip x-bar
        # DMA-transpose fragments into ~330B packets and starves every other
        # stream for ~65us. Two halves so the first s-tiles unblock early.
        # (Keeping q on the scalar ring concurrent with vwef measured FASTER
        # overall than prioritizing vwef: deferring q pushes its traffic into
        # the collective's window and the mesh data phase doubles.)
        q_r = q_t.rearrange("(c p) s -> p c s", p=P)
        nc.scalar.dma_start(out=qTraw[:, :, 0:SH // 2], in_=q_r[:, :, 0:SH // 2])
        nc.scalar.dma_start(out=qTraw[:, :, SH // 2:], in_=q_r[:, :, SH // 2:])

        # ---- PE warmup: ~10 back-to-back matmuls on zeros release the HAM
        # clock-gate (~3.5us of sustained PE activity) before real data lands.
        with tc.tile_pool(name="warm", bufs=1, space="PSUM") as warmp:
            wt = warmp.tile([P, 512], F32, name="warm")
            for i in range(6):
                nc.tensor.matmul(wt, lhsT=zeros_sb[:, 0:P], rhs=zeros_sb,
                                 start=True, stop=True)
            # re-warm right as the first data lands (rhs zeros -> result
            # unused); absorbs into the DMA shadow
            for i in range(4):
                nc.tensor.matmul(wt, lhsT=vwefs[0][:, 0, 0:P], rhs=zeros_sb,
                                 start=True, stop=True)

        # ---- phase B: VP^T = value^T @ We, VF^T = value^T @ Wf over THIS
        # core's half of the sequence, emitted feature-major. Each PSUM bank
        # holds [vp_e | vf_e] for one 128-feature slice; vf's first matmul
        # relies on vp's start=True having cleared the bank's has_written
        # bits, so both halves accumulate independently.
        with tc.tile_pool(name="accp", bufs=4, space="PSUM") as accp:
            vpf_ps = [accp.tile([P, 2, PK], F32, tag="acc", name=f"vpf{e}")
                      for e in range(4)]
            for n in range(NB):
                for r in range(RB):
                    first = (n == 0 and r == 0)
                    last = (n == NB - 1 and r == RB - 1)
                    for e in range(4):
                        lhsT = vwefs[n][:, r, e * P:(e + 1) * P]
                        nc.tensor.matmul(
                            vpf_ps[e][:, 0, :], lhsT=lhsT,
                            rhs=vwefs[n][:, r, D:D + PK],
                            start=first, stop=last, skip_group_check=True)
                        nc.tensor.matmul(
                            vpf_ps[e][:, 1, :], lhsT=lhsT,
                            rhs=vwefs[n][:, r, D + PK:W],
                            start=False, stop=last, skip_group_check=True)
            for e in range(4):
                eng = nc.scalar if e < 2 else nc.vector
                if eng is nc.scalar:
                    nc.scalar.copy(out=vpf_sb[:, e, :, :], in_=vpf_ps[e])
                else:
                    nc.vector.tensor_copy(out=vpf_sb[:, e, :, :], in_=vpf_ps[e])

        # phase-B stream buffers are dead now; release their 32KB/partition
        # for the attention pools
        vwp_cm.__exit__(None, None, None)

        # q-projection PSUM pool created FIRST so its banks are disjoint
        # from khp's: otherwise the first qproj chains false-WAR on the
        # kh/vh drains and the whole q projection slides ~10us.
        qpp_cm = tc.tile_pool(name="qpp", bufs=3, space="PSUM")
        qpp = qpp_cm.__enter__()

        def emit_qchain(st, j):
            ssl = slice(st * 512, (st + 1) * 512)
            qt = qpp.tile([P, 512], F32, tag="qt", name=f"qt{st}_{j}")
            for dc in range(4):
                nc.tensor.matmul(
                    qt, lhsT=wq_sb[:, dc, j * P:(j + 1) * P],
                    rhs=qTraw[:, dc, ssl],
                    start=(dc == 0), stop=(dc == 3))
            nc.scalar.activation(
                out=qsts[st][:, j, :], in_=qt, func=AF.Identity,
                bias=bq_sb[:, j:j + 1])

        # first two chains fill the PE stall while the vpf drains run
        emit_qchain(0, 0)
        emit_qchain(0, 1)

        # khT[e', pk] = Wk^T @ VPT + rank-1 bias rows; PSUM->SBUF drains
        # split scalar||vector so the collective stages earlier
        with tc.tile_pool(name="khp", bufs=2, space="PSUM") as khp:
            for pr in range(4):
                ps_t = khp.tile([P, PK], F32, tag="kh")
                for ec in range(4):
                    nc.tensor.matmul(
                        ps_t, lhsT=wk_sb[:, ec, pr * P:(pr + 1) * P],
                        rhs=vpf_sb[:, ec, 0, :], start=(ec == 0), stop=False)
                nc.tensor.matmul(
                    ps_t, lhsT=wkaug_sb[:, pr * P:(pr + 1) * P],
                    rhs=auge_sb, start=False, stop=True)
                if pr < 2:
                    nc.scalar.copy(out=khT[:, pr, :], in_=ps_t)
                else:
                    nc.vector.tensor_copy(out=khT[:, pr, :], in_=ps_t)

            # vh[pk, dv] = VFT^T @ Wv + rank-1 bias rows (seq-major in pk)
            for ps in range(2):
                ps_t = khp.tile([P, D], F32, tag="kh")
                for ec in range(4):
                    nc.tensor.matmul(
                        ps_t, lhsT=vpf_sb[:, ec, 1, ps * P:(ps + 1) * P],
                        rhs=wv_sb[:, ec, :], start=(ec == 0), stop=False)
                nc.tensor.matmul(
                    ps_t, lhsT=augf_sb[:, ps * P:(ps + 1) * P],
                    rhs=wvaug_sb, start=False, stop=True)
                if ps == 0:
                    nc.scalar.copy(
                        out=vh_sb[:, ps, :, :],
                        in_=ps_t.rearrange("p (h v) -> p h v", h=H))
                else:
                    nc.vector.tensor_copy(
                        out=vh_sb[:, ps, :, :],
                        in_=ps_t.rearrange("p (h v) -> p h v", h=H))

        # ---- pair AllReduce of the half-sequence khT/vh partials (the
        # rank-1 bias rows were halved on the host so the pair sum applies
        # them exactly once). Two collectives: khT lands first so the
        # attention scores+exp pipeline (which needs only khT) can start
        # while the vh exchange is still in flight. Unstage DMAs ride sync
        # so the second doorbell on gpsimd is not blocked behind the first
        # collective's completion wait.
        pairs = [[0, 1], [2, 3], [4, 5], [6, 7]]
        with tc.tile_pool(name="dramb", bufs=2, space="DRAM") as dramb:
            cck_in = dramb.tile([P, 1024], BF16, name="cck_in")
            cck_out = dramb.tile([P, 1024], BF16, name="cck_out")
            ccv_in = dramb.tile([P, 1024], BF16, name="ccv_in")
            ccv_out = dramb.tile([P, 1024], BF16, name="ccv_out")
            nc.sync.dma_start(out=cck_in,
                              in_=khT.rearrange("p a k -> p (a k)"))
            nc.sync.dma_start(out=ccv_in,
                              in_=vh_sb.rearrange("p c h v -> p (c h v)"))
            nc.gpsimd.collective_compute(
                "AllReduce", OP.add, replica_groups=pairs,
                ins=[cck_in.opt()], outs=[cck_out.opt()])
            nc.gpsimd.collective_compute(
                "AllReduce", OP.add, replica_groups=pairs,
                ins=[ccv_in.opt()], outs=[ccv_out.opt()])
            nc.sync.dma_start(out=khT.rearrange("p a k -> p (a k)"),
                              in_=cck_out)
            nc.sync.dma_start(out=vh_sb.rearrange("p c h v -> p (c h v)"),
                              in_=ccv_out)

        # ---- q projection for all 8 s-tiles (28us of PE work hiding the
        # collective). The PSUM->SBUF drain adds bq on the scalar engine.
        if True:
            for st in range(NT):
                for j in range(4):
                    if st == 0 and j < 2:
                        continue
                    emit_qchain(st, j)



        qpp_cm.__exit__(None, None, None)

        # ---- attention: per (s-tile, head-pair j, pk-chunk c) group:
        # scores row-paired (2 MMs), one batched exp over the 2-bank PSUM
        # group, AV + denominator col-paired with accumulation over c,
        # normalize + drains on vector. Wo for s-tile st-1 is interleaved
        # into s-tile st so the PE never waits on the vector engine.
        out_r = out_t.rearrange("(t c p) d -> t p c d", c=4, p=P)
        with (
            tc.tile_pool(name="scp", bufs=2, space="PSUM") as scp,    # 4 banks
            tc.tile_pool(name="nump", bufs=2, space="PSUM") as nump,  # 2 banks
            tc.tile_pool(name="zp", bufs=1, space="PSUM") as zp,      # 1 bank
            tc.tile_pool(name="otp", bufs=1, space="PSUM") as otp,    # 1 bank
            tc.tile_pool(name="epool", bufs=11) as epool,
            tc.tile_pool(name="rzp", bufs=2) as rzp,
            tc.tile_pool(name="avp", bufs=2) as avp,
            tc.tile_pool(name="ostage", bufs=2) as ostage,
        ):
            # per-group state
            scs = {}
            es = {}
            nzs = {}

            def emit_sc(st, j, c):
                sc = scp.tile([P, 2, 512], F32, tag="sc", name=f"sc{st}_{j}_{c}")
                scs[(st, j, c)] = sc
                csl = slice(c * P, (c + 1) * P)
                nc.tensor.matmul(
                    sc[:, 0, :], lhsT=khT[0:64, j, csl],
                    rhs=qsts[st][0:64, j, :], start=True, stop=True,
                    tile_position=(0, 0))
                nc.tensor.matmul(
                    sc[:, 1, :], lhsT=khT[64:P, j, csl],
                    rhs=qsts[st][64:P, j, :], start=True, stop=True,
                    tile_position=(64, 0))

            def emit_exp(st, j, c):
                sc = scs.pop((st, j, c))
                e2 = epool.tile([P, 1024], BF16, tag="e", name=f"e{st}_{j}_{c}")
                es[(st, j, c)] = e2
                nc.scalar.activation(
                    out=e2, in_=sc.rearrange("p a b -> p (a b)"), func=AF.Exp)

            def emit_avz(st, j, c):
                # AV + denominator for the head pair; accumulate over c.
                e2 = es.pop((st, j, c))
                if c == 0:
                    num = nump.tile([P, 512], F32, tag="num", name=f"nm{st}_{j}")
                    z = zp.tile([P, 512], F32, tag="z", name=f"z{st}_{j}")
                    nzs[(st, j)] = (num, z)
                else:
                    num, z = nzs[(st, j)]
                fl, ll = (c == 0), (c == 1)
                eA, eB = e2[:, 0:512], e2[:, 512:1024]
                nc.tensor.matmul(
                    num[0:64, :], lhsT=vh_sb[:, c, 2 * j, :],
                    rhs=eA, start=fl, stop=ll, tile_position=(0, 0))
                nc.tensor.matmul(
                    num[64:P, :], lhsT=vh_sb[:, c, 2 * j + 1, :],
                    rhs=eB, start=fl, stop=ll, tile_position=(0, 64))
                nc.tensor.matmul(
                    z[0:64, :], lhsT=ones64[:, :],
                    rhs=eA, start=fl, stop=ll, tile_position=(0, 0))
                nc.tensor.matmul(
                    z[64:P, :], lhsT=ones64[:, :],
                    rhs=eB, start=fl, stop=ll, tile_position=(0, 64))

            def emit_norm(st, j, av_sb):
                num, z = nzs.pop((st, j))
                rz = rzp.tile([P, 512], F32, tag="rz", name=f"rz{st}_{j}")
                nc.vector.reciprocal_approx_fast(out=rz, in_=z)
                nc.vector.tensor_tensor(
                    out=av_sb[:, j, :], in0=num, in1=rz, op=OP.mult)

            def emit_wo_chunk(wst, wav, o_sb, sl):
                o_t = otp.tile([P, D], F32, tag="ot", name=f"ot{wst}_{sl}")
                for pr in range(4):
                    nc.tensor.matmul(
                        o_t, lhsT=wav[:, pr, sl * P:(sl + 1) * P],
                        rhs=wo_sb[:, pr, :], start=(pr == 0), stop=(pr == 3))
                nc.vector.tensor_copy(out=o_sb[:, sl, :], in_=o_t)

            # flat software pipeline over all 64 (st, j, c) groups: scores+exp
            # (which need only khT, from the FIRST collective) run LAG groups
            # ahead of AV/Z (which need vh, from the second). Wo chunks of a
            # completed s-tile interleave one per iteration.
            groups = [(st, j, c)
                      for st in range(NT) for j in range(4) for c in range(2)]
            LAG = 9
            avs = {}      # st -> av_sb
            wo_pend = []  # (st, av_sb, o_sb, next_sl)
            avz_ptr = 0
            for i in range(len(groups) + LAG):
                if i < len(groups):
                    st, j, c = groups[i]
                    emit_sc(st, j, c)
                    emit_exp(st, j, c)
                # steady state: one avz per iteration at lag 8; drain phase
                # (all exps emitted): two per iteration so the last s-tile's
                # norm/Wo chain starts earlier
                navz = 1 if i < len(groups) else 2
                for _ in range(navz):
                    if avz_ptr < len(groups) and (
                            i >= len(groups) or avz_ptr <= i - LAG):
                        st, j, c = groups[avz_ptr]
                        avz_ptr += 1
                        if j == 0 and c == 0:
                            avs[st] = avp.tile([P, 4, 512], BF16, tag="av",
                                               name=f"av{st}")
                        emit_avz(st, j, c)
                        if c == 1:
                            emit_norm(st, j, avs[st])
                            if j == 3:
                                o_sb = ostage.tile([P, 4, D], BF16, tag="ost",
                                                   name=f"osb{st}")
                                wo_pend.append([st, avs.pop(st), o_sb, 0])
                if wo_pend and (i % 2 == 1 or i >= len(groups)):
                    wst, wav, wosb, sl = wo_pend[0]
                    emit_wo_chunk(wst, wav, wosb, sl)
                    nc.sync.dma_start(out=out_r[wst, :, sl, :],
                                      in_=wosb[:, sl, :])
                    if sl == 3:
                        wo_pend.pop(0)
                    else:
                        wo_pend[0][3] = sl + 1
            while wo_pend:
                wst, wav, wosb, sl = wo_pend[0]
                emit_wo_chunk(wst, wav, wosb, sl)
                nc.sync.dma_start(out=out_r[wst, :, sl, :], in_=wosb[:, sl, :])
                if sl == 3:
                    wo_pend.pop(0)
                else:
                    wo_pend[0][3] = sl + 1

    nc.finalize()
    return nc


def _prep_inputs(inputs):
    bf = ml_dtypes.bfloat16
    f32 = np.float32
    q = np.ascontiguousarray(inputs["query"])
    v = np.ascontiguousarray(inputs["value"])
    We, Wf = np.asarray(inputs["We"]), np.asarray(inputs["Wf"])
    scale = np.float32(DK ** -0.5)
    ones = np.ones(D, f32)
    sWe = We.astype(f32).sum(0)
    sWf = Wf.astype(f32).sum(0)
    # the rank-1 bias rows are applied on BOTH cores of a batch pair and
    # then pair-AllReduced, so they carry a factor 1/2 here
    # wall: [wq|wk|wv|wo] in the on-chip [P, 4, D] layout ("(c p) e -> p c e")
    def wlayout(w):
        return np.asarray(w).reshape(4, P, D).transpose(1, 0, 2)
    wall = np.concatenate([
        wlayout(np.asarray(inputs["Wq"]) * scale),
        wlayout(inputs["Wk"]),
        wlayout(inputs["Wv"]),
        wlayout(inputs["Wo"]),
    ], axis=1).astype(bf)
    aug = np.concatenate([
        np.stack([np.asarray(inputs["bk"], f32), ones]),
        0.5 * np.stack([sWe, np.asarray(inputs["be"], f32)]),
        np.stack([np.asarray(inputs["bv"], f32), ones]),
        0.5 * np.stack([sWf, np.asarray(inputs["bf"], f32)]),
    ], axis=1).astype(bf)
    shared = {
        "wall": np.ascontiguousarray(wall),
        "aug": np.ascontiguousarray(aug),
        "bq": (np.asarray(inputs["bq"]) * scale).astype(f32),
    }
    in_maps = []
    for c in range(NCORES):
        b, half = c // 2, c % 2
        hs = slice(half * SH, (half + 1) * SH)
        m = dict(shared)
        m["q"] = np.ascontiguousarray(q[b, hs, :].T).astype(bf)
        m["vwef"] = np.ascontiguousarray(np.concatenate(
            [v[b, hs, :], We[hs], Wf[hs]], axis=1)).astype(bf)
        in_maps.append(m)
    return in_maps


def kernel(**inputs):
    if "nc" not in _CACHE:
        _CACHE["nc"] = _build_kernel()
    nc = _CACHE["nc"]
    in_maps = _prep_inputs(inputs)
    res = bass_utils.run_bass_kernel_spmd(nc, in_maps, core_ids=list(range(NCORES)))
    bo = np.asarray(inputs["bo"], np.float32)
    out = np.empty((B, S, D), np.float32)
    for c in range(NCORES):
        b, half = c // 2, c % 2
        out[b, half * SH:(half + 1) * SH, :] = \
            np.asarray(res.results[c]["out"]).astype(np.float32)
    out += bo
    return out


# revision 49
# speedup vs baseline: 1.1264x; 1.1264x over previous
"""Linformer-style multihead attention on 8 Trainium2 NeuronCores.

Shapes (hardcoded): B=4, S=8192, D=512, H=8, DK=DV=64, PK=256.

Sharding: core c handles batch b=c//2, sequence half h=c%2 (4096 query rows).
The Linformer K/V projections contract over the FULL sequence; each core
contracts its own half and the batch-pair AllReduces the tiny khT/vh partials.

Key algebra (reassociation): reference computes k = value@Wk then We^T@k.
We instead compute VP^T = value^T@We (8192-contraction, emitted feature-major
directly so no PE transposes are needed) then khT = Wk^T@VP^T (512-
contraction). Biases fold in as rank-1 augmentation rows of the small
matmuls; bq rides the q-projection drain; bo is applied on the host.

Schedule: ~10 warmup matmuls on zeros keep the PE HAM clock-gate warm from
t~1us (cold PE runs at 1.2GHz vs 2.4 warm). Phase B streams one host-packed
[v|we|wf] tensor in 4x2MB blocks (16KB partition lines) alone on the sync
HWDGE queue; weights ride one packed blob + host-pretransposed q on scalar
(few, big transfers: the ~8 recycled DMA semaphores are shared across
queues, so many small triggers false-serialize cross-queue; the on-chip
x-bar DMA-transpose fragments to ~330B packets and starves everything).
kh/vh partials feed TWO pair-AllReduces (khT first, then vh; each ~10-12us
data vs ~32us combined); the q projection (two chains pulled early to fill
the vpf-drain stall) hides the first, and the attention scores+exp
pipeline - which needs only khT - runs LAG=8 groups ahead of AV/Z so the
PE works through the second. Attention runs per
(s-tile, head-pair, pk-chunk) groups: scores row-paired on the PE, exp
batched as one N=1024 scalar activation spanning two PSUM banks, AV +
denominator col-paired, softmax normalize and PSUM drains on the vector
engine, output stores in bf16. Everything PE-side is bf16.
"""

import numpy as np
import ml_dtypes
from contextlib import ExitStack

import concourse.bass as bass
import concourse.bacc as bacc
import concourse.mybir as mybir
import concourse.tile as tile
from concourse import bass_utils

B, S, D = 4, 8192, 512
H, DK, DV, PK = 8, 64, 64, 256
SH = S // 2  # per-core query rows
NCORES = 8
P = 128

F32 = mybir.dt.float32
BF16 = mybir.dt.bfloat16
AF = mybir.ActivationFunctionType
OP = mybir.AluOpType

_CACHE = {}

NB = 4          # phase-B n-blocks of 1024 rows
RB = 8          # contiguous rows per partition line
NT = SH // 512  # 8 s-tiles of 512 query rows


def _build_kernel():
    nc = bacc.Bacc(
        trn_type="TRN2",
        target_bir_lowering=False,
        debug=False,
        num_devices=NCORES,
    )

    # vwef: host-packed [v | we | wf] rows (1024 cols) so each phase-B block
    # is ONE 2MB DMA; wall: all [P,4,D]-layout weights (wq,wk,wv,wo) packed;
    # aug: the 4 rank-augmentation row-pairs packed.
    q_t = nc.dram_tensor("q", [D, SH], BF16, kind="ExternalInput").ap()
    vwef_t = nc.dram_tensor("vwef", [SH, D + 2 * PK], BF16,
                            kind="ExternalInput").ap()
    wall_t = nc.dram_tensor("wall", [P, 16, D], BF16, kind="ExternalInput").ap()
    aug_t = nc.dram_tensor("aug", [2, 2 * D + 2 * PK], BF16,
                           kind="ExternalInput").ap()
    bq_t = nc.dram_tensor("bq", [D], F32, kind="ExternalInput").ap()
    out_t = nc.dram_tensor("out", [SH, D], BF16, kind="ExternalOutput").ap()

    with ExitStack() as ctx:
        tc = ctx.enter_context(tile.TileContext(nc))
        consts = ctx.enter_context(tc.tile_pool(name="consts", bufs=1))
        big = ctx.enter_context(tc.tile_pool(name="big", bufs=1))

        # ---- persistent activations ----
        qTraw = big.tile([P, 4, SH], BF16)     # query, feature-major
        khT = big.tile([P, 4, PK], BF16)       # [dk(2 heads/row-block), pair, pk]
        vh_sb = big.tile([P, 2, H, DV], BF16)  # [pk rows, chunk, head, dv]
        vpf_sb = big.tile([P, 4, 2, PK], BF16)  # [e-in-slice, e-slice, vp|vf, pk]
        qsts = [big.tile([P, 4, 512], BF16, name=f"qst{st}") for st in range(NT)]

        # ---- constants / weights ----
        wall_sb = consts.tile([P, 16, D], BF16)
        wq_sb = wall_sb[:, 0:4, :]
        wk_sb = wall_sb[:, 4:8, :]
        wv_sb = wall_sb[:, 8:12, :]
        wo_sb = wall_sb[:, 12:16, :]
        aug_sb = consts.tile([2, 2 * D + 2 * PK], BF16)
        wkaug_sb = aug_sb[:, 0:D]
        auge_sb = aug_sb[:, D:D + PK]
        wvaug_sb = aug_sb[:, D + PK:2 * D + PK]
        augf_sb = aug_sb[:, 2 * D + PK:2 * D + 2 * PK]
        bq_sb = consts.tile([P, 4], F32)
        ones64 = consts.tile([P, 64], BF16)
        zeros_sb = consts.tile([P, 512], BF16)
        nc.gpsimd.memset(ones64, 1.0)
        nc.gpsimd.memset(zeros_sb, 0.0)

        # ---- DMA kickoff (all streams fire immediately) ----
        # Packed phase-B stream alone on sync (4 x 2MB, 16KB partition
        # lines); weights + host-pretransposed q halves on scalar. Few, big
        # transfers: the ~8 recycled DMA semaphore slots are shared across
        # queues, so many small triggers false-serialize cross-queue.
        vwef_r = vwef_t.rearrange("(n p r) d -> p n (r d)", p=P, r=RB)
        W = D + 2 * PK
        vwp_cm = tc.tile_pool(name="vwp", bufs=1)
        vwp = vwp_cm.__enter__()
        vwefs = []
        # blocks 0-2 on sync; block 3 on scalar (after the weights, before
        # q): the rings drain in parallel so the LAST-consumed block is
        # already resident when the sync stream finishes - phase-B's matmul
        # end moves ~8us earlier. Both rings stay continuously busy (an idle
        # ring with deferred q traffic is the known-bad collective pattern).
        for n in range(NB):
            t = vwp.tile([P, RB, W], BF16, name=f"vwef{n}")
            vwefs.append(t)
        for n in range(NB - 1):
            f = vwefs[n].rearrange("p r d -> p (r d)")
            step = RB * W // 4
            for k in range(4):
                nc.sync.dma_start(out=f[:, k * step:(k + 1) * step],
                                  in_=vwef_r[:, n, k * step:(k + 1) * step])
        nc.scalar.dma_start(out=wall_sb, in_=wall_t)
        nc.scalar.dma_start(out=aug_sb, in_=aug_t)
        nc.scalar.dma_start(out=bq_sb, in_=bq_t.rearrange("(c p) -> p c", p=P))
        f3 = vwefs[NB - 1].rearrange("p r d -> p (r d)")
        step = RB * W // 2
        for k in range(2):
            nc.scalar.dma_start(out=f3[:, k * step:(k + 1) * step],
                                in_=vwef_r[:, NB - 1, k * step:(k + 1) * step])
        # q is pre-transposed on the host (feature-major); the on-chip x-bar
        # DMA-transpose fragments into ~330B packets and starves every other
        # stream for ~65us. Two halves so the first s-tiles unblock early.
        # (Keeping q on the scalar ring concurrent with vwef measured FASTER
        # overall than prioritizing vwef: deferring q pushes its traffic into
        # the collective's window and the mesh data phase doubles.)
        q_r = q_t.rearrange("(c p) s -> p c s", p=P)
        nc.scalar.dma_start(out=qTraw[:, :, 0:SH // 2], in_=q_r[:, :, 0:SH // 2])
        nc.scalar.dma_start(out=qTraw[:, :, SH // 2:], in_=q_r[:, :, SH // 2:])

        # ---- PE warmup: ~10 back-to-back matmuls on zeros release the HAM
        # clock-gate (~3.5us of sustained PE activity) before real data lands.
        with tc.tile_pool(name="warm", bufs=1, space="PSUM") as warmp:
            wt = warmp.tile([P, 512], F32, name="warm")
            for i in range(6):
                nc.tensor.matmul(wt, lhsT=zeros_sb[:, 0:P], rhs=zeros_sb,
                                 start=True, stop=True)
            # re-warm right as the first data lands (rhs zeros -> result
            # unused); absorbs into the DMA shadow
            for i in range(4):
                nc.tensor.matmul(wt, lhsT=vwefs[0][:, 0, 0:P], rhs=zeros_sb,
                                 start=True, stop=True)

        # ---- phase B: VP^T = value^T @ We, VF^T = value^T @ Wf over THIS
        # core's half of the sequence, emitted feature-major. Each PSUM bank
        # holds [vp_e | vf_e] for one 128-feature slice; vf's first matmul
        # relies on vp's start=True having cleared the bank's has_written
        # bits, so both halves accumulate independently.
        with tc.tile_pool(name="accp", bufs=4, space="PSUM") as accp:
            vpf_ps = [accp.tile([P, 2, PK], F32, tag="acc", name=f"vpf{e}")
                      for e in range(4)]
            for n in range(NB):
                for r in range(RB):
                    first = (n == 0 and r == 0)
                    last = (n == NB - 1 and r == RB - 1)
                    for e in range(4):
                        lhsT = vwefs[n][:, r, e * P:(e + 1) * P]
                        nc.tensor.matmul(
                            vpf_ps[e][:, 0, :], lhsT=lhsT,
                            rhs=vwefs[n][:, r, D:D + PK],
                            start=first, stop=last, skip_group_check=True)
                        nc.tensor.matmul(
                            vpf_ps[e][:, 1, :], lhsT=lhsT,
                            rhs=vwefs[n][:, r, D + PK:W],
                            start=False, stop=last, skip_group_check=True)
            for e in range(4):
                eng = nc.scalar if e < 2 else nc.vector
                if eng is nc.scalar:
                    nc.scalar.copy(out=vpf_sb[:, e, :, :], in_=vpf_ps[e])
                else:
                    nc.vector.tensor_copy(out=vpf_sb[:, e, :, :], in_=vpf_ps[e])

        # phase-B stream buffers are dead now; release their 32KB/partition
        # for the attention pools
        vwp_cm.__exit__(None, None, None)

        # q-projection PSUM pool created FIRST so its banks are disjoint
        # from khp's: otherwise the first qproj chains false-WAR on the
        # kh/vh drains and the whole q projection slides ~10us.
        qpp_cm = tc.tile_pool(name="qpp", bufs=3, space="PSUM")
        qpp = qpp_cm.__enter__()

        def emit_qchain(st, j):
            ssl = slice(st * 512, (st + 1) * 512)
            qt = qpp.tile([P, 512], F32, tag="qt", name=f"qt{st}_{j}")
            for dc in range(4):
                nc.tensor.matmul(
                    qt, lhsT=wq_sb[:, dc, j * P:(j + 1) * P],
                    rhs=qTraw[:, dc, ssl],
                    start=(dc == 0), stop=(dc == 3))
            nc.scalar.activation(
                out=qsts[st][:, j, :], in_=qt, func=AF.Identity,
                bias=bq_sb[:, j:j + 1])

        # first two chains fill the PE stall while the vpf drains run
        emit_qchain(0, 0)
        emit_qchain(0, 1)

        # khT[e', pk] = Wk^T @ VPT + rank-1 bias rows; PSUM->SBUF drains
        # split scalar||vector so the collective stages earlier
        with tc.tile_pool(name="khp", bufs=2, space="PSUM") as khp:
            for pr in range(4):
                ps_t = khp.tile([P, PK], F32, tag="kh")
                for ec in range(4):
                    nc.tensor.matmul(
                        ps_t, lhsT=wk_sb[:, ec, pr * P:(pr + 1) * P],
                        rhs=vpf_sb[:, ec, 0, :], start=(ec == 0), stop=False)
                nc.tensor.matmul(
                    ps_t, lhsT=wkaug_sb[:, pr * P:(pr + 1) * P],
                    rhs=auge_sb, start=False, stop=True)
                if pr < 2:
                    nc.scalar.copy(out=khT[:, pr, :], in_=ps_t)
                else:
                    nc.vector.tensor_copy(out=khT[:, pr, :], in_=ps_t)

            # vh[pk, dv] = VFT^T @ Wv + rank-1 bias rows (seq-major in pk)
            for ps in range(2):
                ps_t = khp.tile([P, D], F32, tag="kh")
                for ec in range(4):
                    nc.tensor.matmul(
                        ps_t, lhsT=vpf_sb[:, ec, 1, ps * P:(ps + 1) * P],
                        rhs=wv_sb[:, ec, :], start=(ec == 0), stop=False)
                nc.tensor.matmul(
                    ps_t, lhsT=augf_sb[:, ps * P:(ps + 1) * P],
                    rhs=wvaug_sb, start=False, stop=True)
                if ps == 0:
                    nc.scalar.copy(
                        out=vh_sb[:, ps, :, :],
                        in_=ps_t.rearrange("p (h v) -> p h v", h=H))
                else:
                    nc.vector.tensor_copy(
                        out=vh_sb[:, ps, :, :],
                        in_=ps_t.rearrange("p (h v) -> p h v", h=H))

        # ---- pair AllReduce of the half-sequence khT/vh partials (the
        # rank-1 bias rows were halved on the host so the pair sum applies
        # them exactly once). Two collectives: khT lands first so the
        # attention scores+exp pipeline (which needs only khT) can start
        # while the vh exchange is still in flight. Unstage DMAs ride sync
        # so the second doorbell on gpsimd is not blocked behind the first
        # collective's completion wait.
        pairs = [[0, 1], [2, 3], [4, 5], [6, 7]]
        with tc.tile_pool(name="dramb", bufs=2, space="DRAM") as dramb:
            cck_in = dramb.tile([P, 1024], BF16, name="cck_in")
            cck_out = dramb.tile([P, 1024], BF16, name="cck_out")
            ccv_in = dramb.tile([P, 1024], BF16, name="ccv_in")
            ccv_out = dramb.tile([P, 1024], BF16, name="ccv_out")
            nc.sync.dma_start(out=cck_in,
                              in_=khT.rearrange("p a k -> p (a k)"))
            nc.sync.dma_start(out=ccv_in,
                              in_=vh_sb.rearrange("p c h v -> p (c h v)"))
            nc.gpsimd.collective_compute(
                "AllReduce", OP.add, replica_groups=pairs,
                ins=[cck_in.opt()], outs=[cck_out.opt()])
            nc.gpsimd.collective_compute(
                "AllReduce", OP.add, replica_groups=pairs,
                ins=[ccv_in.opt()], outs=[ccv_out.opt()])
            nc.sync.dma_start(out=khT.rearrange("p a k -> p (a k)"),
                              in_=cck_out)
            nc.sync.dma_start(out=vh_sb.rearrange("p c h v -> p (c h v)"),
                              in_=ccv_out)

        # ---- q projection for all 8 s-tiles (28us of PE work hiding the
        # collective). The PSUM->SBUF drain adds bq on the scalar engine.
        if True:
            for st in range(NT):
                for j in range(4):
                    if st == 0 and j < 2:
                        continue
                    emit_qchain(st, j)



        qpp_cm.__exit__(None, None, None)

        # ---- attention: per (s-tile, head-pair j, pk-chunk c) group:
        # scores row-paired (2 MMs), one batched exp over the 2-bank PSUM
        # group, AV + denominator col-paired with accumulation over c,
        # normalize + drains on vector. Wo for s-tile st-1 is interleaved
        # into s-tile st so the PE never waits on the vector engine.
        out_r = out_t.rearrange("(t c p) d -> t p c d", c=4, p=P)
        with (
            tc.tile_pool(name="scp", bufs=2, space="PSUM") as scp,    # 4 banks
            tc.tile_pool(name="nump", bufs=2, space="PSUM") as nump,  # 2 banks
            tc.tile_pool(name="zp", bufs=1, space="PSUM") as zp,      # 1 bank
            tc.tile_pool(name="otp", bufs=1, space="PSUM") as otp,    # 1 bank
            tc.tile_pool(name="epool", bufs=10) as epool,
            tc.tile_pool(name="rzp", bufs=2) as rzp,
            tc.tile_pool(name="avp", bufs=2) as avp,
            tc.tile_pool(name="ostage", bufs=2) as ostage,
        ):
            # per-group state
            scs = {}
            es = {}
            nzs = {}

            def emit_sc(st, j, c):
                sc = scp.tile([P, 2, 512], F32, tag="sc", name=f"sc{st}_{j}_{c}")
                scs[(st, j, c)] = sc
                csl = slice(c * P, (c + 1) * P)
                nc.tensor.matmul(
                    sc[:, 0, :], lhsT=khT[0:64, j, csl],
                    rhs=qsts[st][0:64, j, :], start=True, stop=True,
                    tile_position=(0, 0))
                nc.tensor.matmul(
                    sc[:, 1, :], lhsT=khT[64:P, j, csl],
                    rhs=qsts[st][64:P, j, :], start=True, stop=True,
                    tile_position=(64, 0))

            def emit_exp(st, j, c):
                sc = scs.pop((st, j, c))
                e2 = epool.tile([P, 1024], BF16, tag="e", name=f"e{st}_{j}_{c}")
                es[(st, j, c)] = e2
                nc.scalar.activation(
                    out=e2, in_=sc.rearrange("p a b -> p (a b)"), func=AF.Exp)

            def emit_avz(st, j, c):
                # AV + denominator for the head pair; accumulate over c.
                e2 = es.pop((st, j, c))
                if c == 0:
                    num = nump.tile([P, 512], F32, tag="num", name=f"nm{st}_{j}")
                    z = zp.tile([P, 512], F32, tag="z", name=f"z{st}_{j}")
                    nzs[(st, j)] = (num, z)
                else:
                    num, z = nzs[(st, j)]
                fl, ll = (c == 0), (c == 1)
                eA, eB = e2[:, 0:512], e2[:, 512:1024]
                nc.tensor.matmul(
                    num[0:64, :], lhsT=vh_sb[:, c, 2 * j, :],
                    rhs=eA, start=fl, stop=ll, tile_position=(0, 0))
                nc.tensor.matmul(
                    num[64:P, :], lhsT=vh_sb[:, c, 2 * j + 1, :],
                    rhs=eB, start=fl, stop=ll, tile_position=(0, 64))
                nc.tensor.matmul(
                    z[0:64, :], lhsT=ones64[:, :],
                    rhs=eA, start=fl, stop=ll, tile_position=(0, 0))
                nc.tensor.matmul(
                    z[64:P, :], lhsT=ones64[:, :],
                    rhs=eB, start=fl, stop=ll, tile_position=(0, 64))

            def emit_norm(st, j, av_sb):
                num, z = nzs.pop((st, j))
                rz = rzp.tile([P, 512], F32, tag="rz", name=f"rz{st}_{j}")
                nc.vector.reciprocal_approx_fast(out=rz, in_=z)
                nc.vector.tensor_tensor(
                    out=av_sb[:, j, :], in0=num, in1=rz, op=OP.mult)

            def emit_wo_chunk(wst, wav, o_sb, sl):
                o_t = otp.tile([P, D], F32, tag="ot", name=f"ot{wst}_{sl}")
                for pr in range(4):
                    nc.tensor.matmul(
                        o_t, lhsT=wav[:, pr, sl * P:(sl + 1) * P],
                        rhs=wo_sb[:, pr, :], start=(pr == 0), stop=(pr == 3))
                nc.vector.tensor_copy(out=o_sb[:, sl, :], in_=o_t)

            # flat software pipeline over all 64 (st, j, c) groups: scores+exp
            # (which need only khT, from the FIRST collective) run LAG groups
            # ahead of AV/Z (which need vh, from the second). Wo chunks of a
            # completed s-tile interleave one per iteration.
            groups = [(st, j, c)
                      for st in range(NT) for j in range(4) for c in range(2)]
            LAG = 8
            avs = {}      # st -> av_sb
            wo_pend = []  # (st, av_sb, o_sb, next_sl)
            avz_ptr = 0
            for i in range(len(groups) + LAG):
                if i < len(groups):
                    st, j, c = groups[i]
                    emit_sc(st, j, c)
                    emit_exp(st, j, c)
                # steady state: one avz per iteration at lag 8; drain phase
                # (all exps emitted): two per iteration so the last s-tile's
                # norm/Wo chain starts earlier
                navz = 1 if i < len(groups) else 2
                for _ in range(navz):
                    if avz_ptr < len(groups) and (
                            i >= len(groups) or avz_ptr <= i - LAG):
                        st, j, c = groups[avz_ptr]
                        avz_ptr += 1
                        if j == 0 and c == 0:
                            avs[st] = avp.tile([P, 4, 512], BF16, tag="av",
                                               name=f"av{st}")
                        emit_avz(st, j, c)
                        if c == 1:
                            emit_norm(st, j, avs[st])
                            if j == 3:
                                o_sb = ostage.tile([P, 4, D], BF16, tag="ost",
                                                   name=f"osb{st}")
                                wo_pend.append([st, avs.pop(st), o_sb, 0])
                if wo_pend and (i % 2 == 1 or i >= len(groups)):
                    wst, wav, wosb, sl = wo_pend[0]
                    emit_wo_chunk(wst, wav, wosb, sl)
                    nc.sync.dma_start(out=out_r[wst, :, sl, :],
                                      in_=wosb[:, sl, :])
                    if sl == 3:
                        wo_pend.pop(0)
                    else:
                        wo_pend[0][3] = sl + 1
            while wo_pend:
                wst, wav, wosb, sl = wo_pend[0]
                emit_wo_chunk(wst, wav, wosb, sl)
                nc.sync.dma_start(out=out_r[wst, :, sl, :], in_=wosb[:, sl, :])
                if sl == 3:
                    wo_pend.pop(0)
                else:
                    wo_pend[0][3] = sl + 1

    nc.finalize()
    return nc


def _prep_inputs(inputs):
    bf = ml_dtypes.bfloat16
    f32 = np.float32
    q = np.ascontiguousarray(inputs["query"])
    v = np.ascontiguousarray(inputs["value"])
    We, Wf = np.asarray(inputs["We"]), np.asarray(inputs["Wf"])
    scale = np.float32(DK ** -0.5)
    ones = np.ones(D, f32)
    sWe = We.astype(f32).sum(0)
    sWf = Wf.astype(f32).sum(0)
    # the rank-1 bias rows are applied on BOTH cores of a batch pair and
    # then pair-AllReduced, so they carry a factor 1/2 here
    # wall: [wq|wk|wv|wo] in the on-chip [P, 4, D] layout ("(c p) e -> p c e")
    def wlayout(w):
        return np.asarray(w).reshape(4, P, D).transpose(1, 0, 2)
    wall = np.concatenate([
        wlayout(np.asarray(inputs["Wq"]) * scale),
        wlayout(inputs["Wk"]),
        wlayout(inputs["Wv"]),
        wlayout(inputs["Wo"]),
    ], axis=1).astype(bf)
    aug = np.concatenate([
        np.stack([np.asarray(inputs["bk"], f32), ones]),
        0.5 * np.stack([sWe, np.asarray(inputs["be"], f32)]),
        np.stack([np.asarray(inputs["bv"], f32), ones]),
        0.5 * np.stack([sWf, np.asarray(inputs["bf"], f32)]),
    ], axis=1).astype(bf)
    shared = {
        "wall": np.ascontiguousarray(wall),
        "aug": np.ascontiguousarray(aug),
        "bq": (np.asarray(inputs["bq"]) * scale).astype(f32),
    }
    in_maps = []
    for c in range(NCORES):
        b, half = c // 2, c % 2
        hs = slice(half * SH, (half + 1) * SH)
        m = dict(shared)
        m["q"] = np.ascontiguousarray(q[b, hs, :].T).astype(bf)
        m["vwef"] = np.ascontiguousarray(np.concatenate(
            [v[b, hs, :], We[hs], Wf[hs]], axis=1)).astype(bf)
        in_maps.append(m)
    return in_maps


def kernel(**inputs):
    if "nc" not in _CACHE:
        _CACHE["nc"] = _build_kernel()
    nc = _CACHE["nc"]
    in_maps = _prep_inputs(inputs)
    res = bass_utils.run_bass_kernel_spmd(nc, in_maps, core_ids=list(range(NCORES)))
    bo = np.asarray(inputs["bo"], np.float32)
    out = np.empty((B, S, D), np.float32)
    for c in range(NCORES):
        b, half = c // 2, c % 2
        out[b, half * SH:(half + 1) * SH, :] = \
            np.asarray(res.results[c]["out"]).astype(np.float32)
    out += bo
    return out
